# revision 10
# baseline (speedup 1.0000x reference)
"""Trainium2 Bass kernel for nn_Attention_40261023433214 (retrieval_knn).

Computation (per image):
  q = conv1x1(feat_edit, wq, bq); k = conv1x1(feat_ori, wk, bk)
  qu = unfold(q, 16); ku = unfold(k, 16); ku normalized per patch
  energy_T[m, n] = qu[m] . ku_norm[n]   (q-norm skipped: positive per-m scale
                                         doesn't change argmax/argmin over n)
  am = argmax_n energy_T; an = argmin_n
  out = fold(unfold(x1)[am]) + gamma2 * fold(unfold(x2)[an])

The axon tunnel moves ~75 MB/s and the host has ONE cpu, so wall time is
dominated by wire bytes + serial numpy, not device FLOPs.  Split:
  host:   1x1 convs (3 FMAs/pixel) -> q,k at 1/3 the input bytes; unfold +
          transpose to [(r,s), n] layout; fp16 cast (wire format); after the
          device round trip, an exact fp32 re-rank of the device's top-8 for
          the few gap-ambiguous queries, then patch gather + fold from x1/x2.
  device: the compute core -- per image a [1024,256]x[256,1024] energy
          matmul (PE, fp16 in / fp32 accum) + row top-8 via DVE Max8/MaxIndex,
          data-parallel 4 images per core on 8 cores.
Wire: 33.5 MB of fp16 q^T/k^T in (pipelined behind the conv/pack via async
device_put), ~0.9 MB of top-8 index/value tables back.

fp16 safety: |fp16 energy - fp32 energy| <= 2^-10 * ||q_m||  (q and k are
elementwise fp16-rounded, |sum k*dq| <= 2^-11 ||q|| ||k_n||=1, same for dk).
A query needs exact re-ranking only when its device top-2 gap is below
2^-9*||q_m|| (x1.5 safety used).  The true argmax falling outside the
device top-8 would need 8 keys within that margin -- measured 0/32768 with
huge margin on the reference input distribution.
"""
import sys
sys.path.insert(0, '/opt/trn_rl_repo')
import numpy as np

B, C, H, W = 32, 3, 512, 512
KP = 16                 # patch size
NB = H // KP            # 32 patch blocks per side
N = NB * NB             # 1024 patches
PD = KP * KP            # 256 positions per patch (single channel)
N_CORES = 8
IPC = B // N_CORES      # 4 images per core
THETA = 3.0 * 2.0 ** -10   # re-rank gate: 1.5x the 2^-9 fp16 gap bound

_CACHE = {}


def _build():
    import concourse.bass as bass
    import concourse.mybir as mybir
    from concourse.tile import TileContext

    F32 = mybir.dt.float32
    F16 = mybir.dt.float16
    U16 = mybir.dt.uint16

    nc = bass.Bass()
    qT_d = nc.declare_dram_parameter("qT", [IPC, PD, N], F16, isOutput=False)
    kT_d = nc.declare_dram_parameter("kT", [IPC, PD, N], F16, isOutput=False)
    am8_d = nc.declare_dram_parameter("am8", [IPC, N, 8], U16, isOutput=True)
    mx2_d = nc.declare_dram_parameter("mx2", [IPC, N, 2], F32, isOutput=True)
    an8_d = nc.declare_dram_parameter("an8", [IPC, N, 8], U16, isOutput=True)
    mn2_d = nc.declare_dram_parameter("mn2", [IPC, N, 2], F32, isOutput=True)

    am8_v = am8_d.rearrange("i (t p) e -> i t p e", p=128)
    mx2_v = mx2_d.rearrange("i (t p) e -> i t p e", p=128)
    an8_v = an8_d.rearrange("i (t p) e -> i t p e", p=128)
    mn2_v = mn2_d.rearrange("i (t p) e -> i t p e", p=128)

    with TileContext(nc) as tc:
        with (
            tc.tile_pool(name="qk", bufs=8) as qkp,
            tc.tile_pool(name="esb", bufs=4) as esbp,
            tc.tile_pool(name="tiny", bufs=8) as tinyp,
            tc.tile_pool(name="idx", bufs=8) as idxp,
            tc.tile_pool(name="ps", bufs=4, space="PSUM") as psp,
        ):
            for b in range(IPC):
                qlo = qkp.tile([128, N], F16, name="qlo", tag="qk")
                qhi = qkp.tile([128, N], F16, name="qhi", tag="qk")
                klo = qkp.tile([128, N], F16, name="klo", tag="qk")
                khi = qkp.tile([128, N], F16, name="khi", tag="qk")
                nc.sync.dma_start(out=qlo[:], in_=qT_d[b, 0:128, :])
                nc.scalar.dma_start(out=qhi[:], in_=qT_d[b, 128:256, :])
                nc.sync.dma_start(out=klo[:], in_=kT_d[b, 0:128, :])
                nc.scalar.dma_start(out=khi[:], in_=kT_d[b, 128:256, :])
                for mt in range(8):
                    esb = esbp.tile([128, N], F32, name="esb", tag="esb")
                    for nf in range(2):
                        pe = psp.tile([128, 512], F32, name="pe", tag="ps", space="PSUM")
                        nc.tensor.matmul(pe[:], qlo[:, 128 * mt:128 * (mt + 1)],
                                         klo[:, 512 * nf:512 * (nf + 1)], start=True, stop=False)
                        nc.tensor.matmul(pe[:], qhi[:, 128 * mt:128 * (mt + 1)],
                                         khi[:, 512 * nf:512 * (nf + 1)], start=False, stop=True)
                        nc.scalar.copy(esb[:, 512 * nf:512 * (nf + 1)], pe[:])
                    mx = tinyp.tile([128, 8], F32, name="mx", tag="tiny")
                    ix = idxp.tile([128, 8], U16, name="ix", tag="idx")
                    nc.vector.max(mx[:], esb[:])
                    nc.vector.max_index(ix[:], mx[:], esb[:])
                    nc.sync.dma_start(out=am8_v[b, mt], in_=ix[:])
                    nc.sync.dma_start(out=mx2_v[b, mt], in_=mx[:, 0:2])
                    esn = esbp.tile([128, N], F32, name="esn", tag="esb")
                    nc.scalar.mul(esn[:], esb[:], -1.0)
                    mn = tinyp.tile([128, 8], F32, name="mn", tag="tiny")
                    inx = idxp.tile([128, 8], U16, name="inx", tag="idx")
                    nc.vector.max(mn[:], esn[:])
                    nc.vector.max_index(inx[:], mn[:], esn[:])
                    nc.scalar.dma_start(out=an8_v[b, mt], in_=inx[:])
                    nc.scalar.dma_start(out=mn2_v[b, mt], in_=mn[:, 0:2])

    # wait-splitting post-pass (walrus in this container allows 1 sync-wait/inst)
    for f in nc.m.functions:
        for blk in f.blocks:
            newlist = []
            for i in blk.instructions:
                si = i.sync_info
                if si is not None and len(si.on_wait) > 1:
                    waits = list(si.on_wait)
                    keep = waits[-1:]
                    rest = waits[:-1]
                    for j, wchunk in enumerate(rest):
                        nop = mybir.InstNoOp(name=f"{i.name}-ws-{j}", ins=[], outs=[])
                        nop.engine = i.engine
                        nop.sync_info = mybir.SyncInfo(on_wait=[wchunk], on_update=[])
                        newlist.append(nop)
                    si.on_wait = keep
                newlist.append(i)
            blk.instructions[:] = newlist
    return nc


def _get_runner():
    """Cached jitted SPMD runner + input sharding for async device_put."""
    if "runner" in _CACHE:
        return _CACHE["runner"]
    import jax
    import concourse.mybir as mybir
    from concourse import bass2jax
    from jax.experimental.shard_map import shard_map
    from jax.sharding import Mesh, PartitionSpec, NamedSharding

    nc = _build()
    bass2jax.install_neuronx_cc_hook()

    partition_name = nc.partition_id_tensor.name if nc.partition_id_tensor else None
    in_names, out_names, out_avals = [], [], []
    for alloc in nc.m.functions[0].allocations:
        if not isinstance(alloc, mybir.MemoryLocationSet):
            continue
        name = alloc.memorylocations[0].name
        if alloc.kind == "ExternalInput":
            if name != partition_name:
                in_names.append(name)
        elif alloc.kind == "ExternalOutput":
            out_names.append(name)
            out_avals.append(jax.core.ShapedArray(tuple(alloc.tensor_shape),
                                                  mybir.dt.np(alloc.dtype)))
    n_params = len(in_names)
    n_outs = len(out_avals)
    all_in_names = list(in_names) + list(out_names)
    if partition_name is not None:
        all_in_names.append(partition_name)

    def _body(*args):
        operands = list(args)
        if partition_name is not None:
            operands.append(bass2jax.partition_id_tensor())
        outs = bass2jax._bass_exec_p.bind(
            *operands,
            out_avals=tuple(out_avals),
            in_names=tuple(all_in_names),
            out_names=tuple(out_names),
            lowering_input_output_aliases=(),
            sim_require_finite=True,
            sim_require_nnan=True,
            nc=nc,
        )
        return tuple(outs)

    devices = jax.devices()[:N_CORES]
    mesh = Mesh(np.asarray(devices), ("core",))
    donate = tuple(range(n_params, n_params + n_outs))
    sharded = jax.jit(
        shard_map(_body, mesh=mesh,
                  in_specs=(PartitionSpec("core"),) * (n_params + n_outs),
                  out_specs=(PartitionSpec("core"),) * n_outs,
                  check_rep=False),
        donate_argnums=donate, keep_unused=True,
    )
    sharding = NamedSharding(mesh, PartitionSpec("core"))
    zero_shapes = [(N_CORES * a.shape[0], *a.shape[1:]) for a in out_avals]
    zero_dtypes = [a.dtype for a in out_avals]
    make_zeros = jax.jit(
        lambda: tuple(jax.numpy.zeros(s, d) for s, d in zip(zero_shapes, zero_dtypes)),
        out_shardings=(sharding,) * n_outs,
    )
    runner = (sharded, make_zeros, in_names, out_names, sharding, jax)
    _CACHE["runner"] = runner
    return runner


def _conv1x1(feat, w, bias):
    # feat: [B,C,H,W], w: [C], bias: scalar -> [B,H,W]
    out = np.einsum('bcx,c->bx', feat.reshape(B, C, H * W), w.astype(np.float32))
    out += np.float32(bias)
    return out.reshape(B, H, W)


def _pack_T(img):
    # [B,H,W] -> [B, (r,s)=256, (hb,wb)=1024]
    v = img.reshape(B, NB, KP, NB, KP)
    return np.ascontiguousarray(v.transpose(0, 2, 4, 1, 3)).reshape(B, PD, N)


def _pack_patches(x):
    # [B,C,H,W] -> [B, N, C*256] patch-major rows
    v = x.reshape(B, C, NB, KP, NB, KP)
    return np.ascontiguousarray(v.transpose(0, 2, 4, 1, 3, 5)).reshape(B, N, C * PD)


def _gather_fold(xp, idx):
    # xp: [B, N, C*256] patch rows; idx: [B, N] -> [B,C,H,W]
    out = np.empty((B, C, H, W), np.float32)
    for b in range(B):
        ov = out[b].reshape(C, NB, KP, NB, KP).transpose(1, 3, 0, 2, 4)
        ov[...] = xp[b, idx[b]].reshape(NB, NB, C, KP, KP)
    return out


def _rerank(sel, top8, gap, qn, qT, kT, knrm, pick_min):
    """Exact fp32 re-rank of gap-ambiguous queries; edits sel in place.

    kT is the RAW fp32 k; candidates are normalized here with the same
    fp32 elementwise divide the reference uses."""
    ib, im = np.nonzero(gap <= THETA * qn)
    if ib.size == 0:
        return 0
    cand = top8[ib, im].astype(np.int64)            # [M,8]
    kv = kT[ib[:, None], :, cand]                   # [M,8,256] raw fp32 k
    kv /= knrm[ib[:, None], cand][:, :, None]
    qv = qT[ib, :, im]                              # [M,256]
    e = np.einsum('mcp,mp->mc', kv, qv)
    best = e.argmin(1) if pick_min else e.argmax(1)
    sel[ib, im] = cand[np.arange(ib.size), best]
    return ib.size


def kernel(**inputs) -> np.ndarray:
    feat_edit = np.asarray(inputs["feat_edit"], dtype=np.float32)
    feat_ori = np.asarray(inputs["feat_ori"], dtype=np.float32)
    x1 = np.asarray(inputs["x1"], dtype=np.float32)
    wq = np.asarray(inputs["wq"], dtype=np.float32).ravel()
    bq = np.asarray(inputs["bq"], dtype=np.float32).ravel()[0]
    wk = np.asarray(inputs["wk"], dtype=np.float32).ravel()
    bk = np.asarray(inputs["bk"], dtype=np.float32).ravel()[0]
    gamma2 = np.float32(np.asarray(inputs["gamma2"], dtype=np.float32).ravel()[0])
    with_x2 = bool(gamma2 != 0.0)

    sharded, make_zeros, in_names, out_names, sharding, jax = _get_runner()

    # pre: conv + pack + fp16 cast, with each tensor's wire transfer started
    # (async device_put) before the next one is computed
    q = _conv1x1(feat_edit, wq, bq)
    qT = _pack_T(q)
    dev = {"qT": jax.device_put(qT.astype(np.float16), sharding)}
    k = _conv1x1(feat_ori, wk, bk)
    kT = _pack_T(k)
    knrm = np.maximum(np.sqrt(np.einsum('bpn,bpn->bn', kT, kT)), np.float32(1e-12))
    k16 = np.empty(kT.shape, np.float16)
    np.multiply(kT, (np.float32(1.0) / knrm)[:, None, :], out=k16, casting='unsafe')
    dev["kT"] = jax.device_put(k16, sharding)
    args = [dev[n] for n in in_names] + list(make_zeros())
    out_arrs = sharded(*args)  # async dispatch

    # overlap window: host work that doesn't need the device results
    x1p = _pack_patches(x1)
    qn = np.sqrt(np.einsum('bpn,bpn->bn', qT, qT))   # for the re-rank gate
    if with_x2:
        x2p = _pack_patches(np.asarray(inputs["x2"], dtype=np.float32))

    am8 = np.asarray(out_arrs[out_names.index("am8")])
    mx2 = np.asarray(out_arrs[out_names.index("mx2")])
    am = am8[:, :, 0].astype(np.int64)
    _rerank(am, am8, mx2[:, :, 0] - mx2[:, :, 1], qn, qT, kT, knrm, pick_min=False)
    out = _gather_fold(x1p, am)
    if with_x2:
        an8 = np.asarray(out_arrs[out_names.index("an8")])
        mn2 = np.asarray(out_arrs[out_names.index("mn2")])
        an = an8[:, :, 0].astype(np.int64)
        _rerank(an, an8, mn2[:, :, 0] - mn2[:, :, 1], qn, qT, kT, knrm, pick_min=True)
        out += gamma2 * _gather_fold(x2p, an)
    return out


# revision 14
# speedup vs baseline: 1.2928x; 1.2928x over previous
"""Trainium2 Bass kernel for nn_Attention_40261023433214 (retrieval_knn).

Computation (per image):
  q = conv1x1(feat_edit, wq, bq); k = conv1x1(feat_ori, wk, bk)
  qu = unfold(q, 16); ku = unfold(k, 16); ku normalized per patch
  energy_T[m, n] = qu[m] . ku_norm[n]   (q-norm skipped: positive per-m scale
                                         doesn't change argmax/argmin over n)
  am = argmax_n energy_T; an = argmin_n
  out = fold(unfold(x1)[am]) + gamma2 * fold(unfold(x2)[an])

The axon tunnel moves ~75 MB/s and the host has ONE cpu, so wall time is
dominated by wire bytes + serial numpy, not device FLOPs.  Split:
  host:   1x1 convs (3 FMAs/pixel) -> q,k at 1/3 the input bytes; unfold +
          transpose to [(r,s), n] layout; k-normalize fused into the fp16
          wire cast; after the device round trip, an exact fp32 re-rank of
          the device's top-8 for the few gap-ambiguous queries, then the
          patch gather + fold from x1/x2 (data already on the host).
  device: the compute core -- per image a [1024,256]x[256,1024] energy
          matmul (PE, fp16 in / fp32 accum) + row top-8 via DVE Max8/MaxIndex,
          data-parallel 4 images per core on 8 cores.
Wire: 33.5 MB of fp16 q^T/k^T in, shipped as 4 half-batch tensors so the
first async device_put starts after ~40 ms of host work and the tunnel
stays busy behind the remaining numpy; ~0.9 MB of top-8 tables back.

fp16 safety: |fp16 energy - fp32 energy| <= 2^-10 * ||q_m||  (q and k are
elementwise fp16-rounded, |sum k*dq| <= 2^-11 ||q|| ||k_n||=1, same for dk).
A query needs exact re-ranking only when its device top-2 gap is below
2^-9*||q_m|| (x1.5 safety used).  The true argmax falling outside the
device top-8 would need 8 keys within that margin -- measured 0/32768 with
huge margin on the reference input distribution.
"""
import sys
sys.path.insert(0, '/opt/trn_rl_repo')
import numpy as np

B, C, H, W = 32, 3, 512, 512
KP = 16                 # patch size
NB = H // KP            # 32 patch blocks per side
N = NB * NB             # 1024 patches
PD = KP * KP            # 256 positions per patch (single channel)
N_CORES = 8
IPC = B // N_CORES      # 4 images per core
HB = B // 2             # host pipeline half-batch
IPH = IPC // 2          # images per core per half
THETA = 3.0 * 2.0 ** -10   # re-rank gate: 1.5x the 2^-9 fp16 gap bound

# With half-split inputs, core c computes images (2c, 2c+1, 16+2c, 16+2c+1)
# as its output rows 4c..4c+3; _ROW_OF_IMG[i] is the gathered-output row
# holding image i.
_ROW_OF_IMG = np.empty(B, np.int64)
for _c in range(N_CORES):
    for _b in range(IPC):
        _img = 2 * _c + _b if _b < IPH else HB + 2 * _c + (_b - IPH)
        _ROW_OF_IMG[_img] = IPC * _c + _b

_CACHE = {}


def _build():
    import concourse.bass as bass
    import concourse.mybir as mybir
    from concourse.tile import TileContext

    F32 = mybir.dt.float32
    F16 = mybir.dt.float16
    U16 = mybir.dt.uint16

    nc = bass.Bass()
    qa_d = nc.declare_dram_parameter("qTa", [IPH, PD, N], F16, isOutput=False)
    qb_d = nc.declare_dram_parameter("qTb", [IPH, PD, N], F16, isOutput=False)
    ka_d = nc.declare_dram_parameter("kTa", [IPH, PD, N], F16, isOutput=False)
    kb_d = nc.declare_dram_parameter("kTb", [IPH, PD, N], F16, isOutput=False)
    am8_d = nc.declare_dram_parameter("am8", [IPC, N, 8], U16, isOutput=True)
    mx2_d = nc.declare_dram_parameter("mx2", [IPC, N, 2], F32, isOutput=True)
    an8_d = nc.declare_dram_parameter("an8", [IPC, N, 8], U16, isOutput=True)
    mn2_d = nc.declare_dram_parameter("mn2", [IPC, N, 2], F32, isOutput=True)

    am8_v = am8_d.rearrange("i (t p) e -> i t p e", p=128)
    mx2_v = mx2_d.rearrange("i (t p) e -> i t p e", p=128)
    an8_v = an8_d.rearrange("i (t p) e -> i t p e", p=128)
    mn2_v = mn2_d.rearrange("i (t p) e -> i t p e", p=128)

    with TileContext(nc) as tc:
        with (
            tc.tile_pool(name="qk", bufs=8) as qkp,
            tc.tile_pool(name="esb", bufs=4) as esbp,
            tc.tile_pool(name="tiny", bufs=8) as tinyp,
            tc.tile_pool(name="idx", bufs=8) as idxp,
            tc.tile_pool(name="ps", bufs=4, space="PSUM") as psp,
        ):
            for b in range(IPC):
                q_d = qa_d if b < IPH else qb_d
                k_d = ka_d if b < IPH else kb_d
                bi = b if b < IPH else b - IPH
                qlo = qkp.tile([128, N], F16, name="qlo", tag="qk")
                qhi = qkp.tile([128, N], F16, name="qhi", tag="qk")
                klo = qkp.tile([128, N], F16, name="klo", tag="qk")
                khi = qkp.tile([128, N], F16, name="khi", tag="qk")
                nc.sync.dma_start(out=qlo[:], in_=q_d[bi, 0:128, :])
                nc.scalar.dma_start(out=qhi[:], in_=q_d[bi, 128:256, :])
                nc.sync.dma_start(out=klo[:], in_=k_d[bi, 0:128, :])
                nc.scalar.dma_start(out=khi[:], in_=k_d[bi, 128:256, :])
                for mt in range(8):
                    esb = esbp.tile([128, N], F32, name="esb", tag="esb")
                    for nf in range(2):
                        pe = psp.tile([128, 512], F32, name="pe", tag="ps", space="PSUM")
                        nc.tensor.matmul(pe[:], qlo[:, 128 * mt:128 * (mt + 1)],
                                         klo[:, 512 * nf:512 * (nf + 1)], start=True, stop=False)
                        nc.tensor.matmul(pe[:], qhi[:, 128 * mt:128 * (mt + 1)],
                                         khi[:, 512 * nf:512 * (nf + 1)], start=False, stop=True)
                        nc.scalar.copy(esb[:, 512 * nf:512 * (nf + 1)], pe[:])
                    mx = tinyp.tile([128, 8], F32, name="mx", tag="tiny")
                    ix = idxp.tile([128, 8], U16, name="ix", tag="idx")
                    nc.vector.max(mx[:], esb[:])
                    nc.vector.max_index(ix[:], mx[:], esb[:])
                    nc.sync.dma_start(out=am8_v[b, mt], in_=ix[:])
                    nc.sync.dma_start(out=mx2_v[b, mt], in_=mx[:, 0:2])
                    esn = esbp.tile([128, N], F32, name="esn", tag="esb")
                    nc.scalar.mul(esn[:], esb[:], -1.0)
                    mn = tinyp.tile([128, 8], F32, name="mn", tag="tiny")
                    inx = idxp.tile([128, 8], U16, name="inx", tag="idx")
                    nc.vector.max(mn[:], esn[:])
                    nc.vector.max_index(inx[:], mn[:], esn[:])
                    nc.scalar.dma_start(out=an8_v[b, mt], in_=inx[:])
                    nc.scalar.dma_start(out=mn2_v[b, mt], in_=mn[:, 0:2])

    # wait-splitting post-pass (walrus in this container allows 1 sync-wait/inst)
    for f in nc.m.functions:
        for blk in f.blocks:
            newlist = []
            for i in blk.instructions:
                si = i.sync_info
                if si is not None and len(si.on_wait) > 1:
                    waits = list(si.on_wait)
                    keep = waits[-1:]
                    rest = waits[:-1]
                    for j, wchunk in enumerate(rest):
                        nop = mybir.InstNoOp(name=f"{i.name}-ws-{j}", ins=[], outs=[])
                        nop.engine = i.engine
                        nop.sync_info = mybir.SyncInfo(on_wait=[wchunk], on_update=[])
                        newlist.append(nop)
                    si.on_wait = keep
                newlist.append(i)
            blk.instructions[:] = newlist
    return nc


def _get_runner():
    """Cached jitted SPMD runner + input sharding for async device_put."""
    if "runner" in _CACHE:
        return _CACHE["runner"]
    import jax
    import concourse.mybir as mybir
    from concourse import bass2jax
    from jax.experimental.shard_map import shard_map
    from jax.sharding import Mesh, PartitionSpec, NamedSharding

    nc = _build()
    bass2jax.install_neuronx_cc_hook()

    partition_name = nc.partition_id_tensor.name if nc.partition_id_tensor else None
    in_names, out_names, out_avals = [], [], []
    for alloc in nc.m.functions[0].allocations:
        if not isinstance(alloc, mybir.MemoryLocationSet):
            continue
        name = alloc.memorylocations[0].name
        if alloc.kind == "ExternalInput":
            if name != partition_name:
                in_names.append(name)
        elif alloc.kind == "ExternalOutput":
            out_names.append(name)
            out_avals.append(jax.core.ShapedArray(tuple(alloc.tensor_shape),
                                                  mybir.dt.np(alloc.dtype)))
    n_params = len(in_names)
    n_outs = len(out_avals)
    all_in_names = list(in_names) + list(out_names)
    if partition_name is not None:
        all_in_names.append(partition_name)

    def _body(*args):
        operands = list(args)
        if partition_name is not None:
            operands.append(bass2jax.partition_id_tensor())
        outs = bass2jax._bass_exec_p.bind(
            *operands,
            out_avals=tuple(out_avals),
            in_names=tuple(all_in_names),
            out_names=tuple(out_names),
            lowering_input_output_aliases=(),
            sim_require_finite=True,
            sim_require_nnan=True,
            nc=nc,
        )
        return tuple(outs)

    devices = jax.devices()[:N_CORES]
    mesh = Mesh(np.asarray(devices), ("core",))
    donate = tuple(range(n_params, n_params + n_outs))
    sharded = jax.jit(
        shard_map(_body, mesh=mesh,
                  in_specs=(PartitionSpec("core"),) * (n_params + n_outs),
                  out_specs=(PartitionSpec("core"),) * n_outs,
                  check_rep=False),
        donate_argnums=donate, keep_unused=True,
    )
    sharding = NamedSharding(mesh, PartitionSpec("core"))
    zero_shapes = [(N_CORES * a.shape[0], *a.shape[1:]) for a in out_avals]
    zero_dtypes = [a.dtype for a in out_avals]
    make_zeros = jax.jit(
        lambda: tuple(jax.numpy.zeros(s, d) for s, d in zip(zero_shapes, zero_dtypes)),
        out_shardings=(sharding,) * n_outs,
    )
    runner = (sharded, make_zeros, in_names, out_names, sharding, jax)
    _CACHE["runner"] = runner
    return runner


def _conv1x1(feat, w, bias):
    # feat: [b,C,H,W], w: [C], bias: scalar -> [b,H,W]
    nb = feat.shape[0]
    out = np.einsum('bcx,c->bx', feat.reshape(nb, C, H * W), w.astype(np.float32))
    out += np.float32(bias)
    return out.reshape(nb, H, W)


def _pack_T(img):
    # [b,H,W] -> [b, (r,s)=256, (hb,wb)=1024]
    nb = img.shape[0]
    v = img.reshape(nb, NB, KP, NB, KP)
    return np.ascontiguousarray(v.transpose(0, 2, 4, 1, 3)).reshape(nb, PD, N)


def _pack_patches(x):
    # [B,C,H,W] -> [B, N, C*256] patch-major rows
    v = x.reshape(B, C, NB, KP, NB, KP)
    return np.ascontiguousarray(v.transpose(0, 2, 4, 1, 3, 5)).reshape(B, N, C * PD)


def _gather_fold(xp, idx):
    # xp: [B, N, C*256] patch rows; idx: [B, N] -> [B,C,H,W]
    out = np.empty((B, C, H, W), np.float32)
    for b in range(B):
        ov = out[b].reshape(C, NB, KP, NB, KP).transpose(1, 3, 0, 2, 4)
        ov[...] = xp[b, idx[b]].reshape(NB, NB, C, KP, KP)
    return out


def _prep_k(feat_half, wk, bk):
    """conv + pack + normalize-fused-fp16-cast for one image half."""
    k = _conv1x1(feat_half, wk, bk)
    kT = _pack_T(k)
    knrm = np.maximum(np.sqrt(np.einsum('bpn,bpn->bn', kT, kT)), np.float32(1e-12))
    k16 = np.empty(kT.shape, np.float16)
    np.multiply(kT, (np.float32(1.0) / knrm)[:, None, :], out=k16, casting='unsafe')
    return kT, knrm, k16


def _rerank(sel, top8, gap, qn, qT, kT, knrm, pick_min):
    """Exact fp32 re-rank of gap-ambiguous queries; edits sel in place.

    kT is the RAW fp32 k; candidates are normalized here with the same
    fp32 elementwise divide the reference uses."""
    ib, im = np.nonzero(gap <= THETA * qn)
    if ib.size == 0:
        return 0
    cand = top8[ib, im].astype(np.int64)            # [M,8]
    kv = kT[ib[:, None], :, cand]                   # [M,8,256] raw fp32 k
    kv /= knrm[ib[:, None], cand][:, :, None]
    qv = qT[ib, :, im]                              # [M,256]
    e = np.einsum('mcp,mp->mc', kv, qv)
    best = e.argmin(1) if pick_min else e.argmax(1)
    sel[ib, im] = cand[np.arange(ib.size), best]
    return ib.size


def kernel(**inputs) -> np.ndarray:
    feat_edit = np.asarray(inputs["feat_edit"], dtype=np.float32)
    feat_ori = np.asarray(inputs["feat_ori"], dtype=np.float32)
    x1 = np.asarray(inputs["x1"], dtype=np.float32)
    wq = np.asarray(inputs["wq"], dtype=np.float32).ravel()
    bq = np.asarray(inputs["bq"], dtype=np.float32).ravel()[0]
    wk = np.asarray(inputs["wk"], dtype=np.float32).ravel()
    bk = np.asarray(inputs["bk"], dtype=np.float32).ravel()[0]
    gamma2 = np.float32(np.asarray(inputs["gamma2"], dtype=np.float32).ravel()[0])
    with_x2 = bool(gamma2 != 0.0)

    sharded, make_zeros, in_names, out_names, sharding, jax = _get_runner()

    # pre: conv + pack + fp16 cast per half-batch; each half's wire transfer
    # (async device_put) starts as soon as it is ready, overlapping the rest
    dev = {}
    qTa = _pack_T(_conv1x1(feat_edit[:HB], wq, bq))
    dev["qTa"] = jax.device_put(qTa.astype(np.float16), sharding)
    qTb = _pack_T(_conv1x1(feat_edit[HB:], wq, bq))
    dev["qTb"] = jax.device_put(qTb.astype(np.float16), sharding)
    kTa, knrma, k16a = _prep_k(feat_ori[:HB], wk, bk)
    dev["kTa"] = jax.device_put(k16a, sharding)
    kTb, knrmb, k16b = _prep_k(feat_ori[HB:], wk, bk)
    dev["kTb"] = jax.device_put(k16b, sharding)
    args = [dev[n] for n in in_names] + list(make_zeros())
    out_arrs = sharded(*args)  # async dispatch

    # overlap window: host work that doesn't need the device results
    x1p = _pack_patches(x1)
    qna = np.sqrt(np.einsum('bpn,bpn->bn', qTa, qTa))   # re-rank gate
    qnb = np.sqrt(np.einsum('bpn,bpn->bn', qTb, qTb))
    if with_x2:
        x2p = _pack_patches(np.asarray(inputs["x2"], dtype=np.float32))

    am8, mx2 = jax.device_get((out_arrs[out_names.index("am8")],
                               out_arrs[out_names.index("mx2")]))
    am8 = am8[_ROW_OF_IMG]
    mx2 = mx2[_ROW_OF_IMG]
    am = am8[:, :, 0].astype(np.int64)
    gap = mx2[:, :, 0] - mx2[:, :, 1]
    _rerank(am[:HB], am8[:HB], gap[:HB], qna, qTa, kTa, knrma, pick_min=False)
    _rerank(am[HB:], am8[HB:], gap[HB:], qnb, qTb, kTb, knrmb, pick_min=False)
    out = _gather_fold(x1p, am)
    if with_x2:
        an8, mn2 = jax.device_get((out_arrs[out_names.index("an8")],
                                   out_arrs[out_names.index("mn2")]))
        an8 = an8[_ROW_OF_IMG]
        mn2 = mn2[_ROW_OF_IMG]
        an = an8[:, :, 0].astype(np.int64)
        gapn = mn2[:, :, 0] - mn2[:, :, 1]
        _rerank(an[:HB], an8[:HB], gapn[:HB], qna, qTa, kTa, knrma, pick_min=True)
        _rerank(an[HB:], an8[HB:], gapn[HB:], qnb, qTb, kTb, knrmb, pick_min=True)
        out += gamma2 * _gather_fold(x2p, an)
    return out


# revision 18
# speedup vs baseline: 1.3714x; 1.0608x over previous
"""Trainium2 Bass kernel for nn_Attention_40261023433214 (retrieval_knn).

Computation (per image):
  q = conv1x1(feat_edit, wq, bq); k = conv1x1(feat_ori, wk, bk)
  qu = unfold(q, 16); ku = unfold(k, 16); ku normalized per patch
  energy_T[m, n] = qu[m] . ku_norm[n]   (q-norm skipped: positive per-m scale
                                         doesn't change argmax/argmin over n)
  am = argmax_n energy_T; an = argmin_n
  out = fold(unfold(x1)[am]) + gamma2 * fold(unfold(x2)[an])

The axon tunnel moves ~75 MB/s and the host has ONE cpu, so wall time is
dominated by wire bytes + serial numpy, not device FLOPs.  Split:
  host:   1x1 convs (3 FMAs/pixel) -> q,k at 1/3 the input bytes; unfold +
          transpose to [(r,s), n] layout; k-normalize fused into the fp16
          wire cast; after the device round trip, an exact fp32 re-rank of
          the device's top-8 for the few gap-ambiguous queries, then the
          patch gather + fold from x1/x2 (data already on the host).
  device: the compute core -- per image a [1024,256]x[256,1024] energy
          matmul (PE, fp16 in / fp32 accum) + row top-8 via DVE Max8/MaxIndex,
          data-parallel 4 images per core on 8 cores.
Wire: 33.5 MB of fp16 q^T/k^T in, shipped as 4 half-batch tensors so the
first async device_put starts after ~40 ms of host work and the tunnel
stays busy behind the remaining numpy; ~0.9 MB of top-8 tables back.

fp16 safety: |fp16 energy - fp32 energy| <= 2^-10 * ||q_m||  (q and k are
elementwise fp16-rounded, |sum k*dq| <= 2^-11 ||q|| ||k_n||=1, same for dk).
A query needs exact re-ranking only when its device top-2 gap is below
2^-9*||q_m|| (x1.5 safety used).  The true argmax falling outside the
device top-8 would need 8 keys within that margin -- measured 0/32768 with
huge margin on the reference input distribution.
"""
import sys
sys.path.insert(0, '/opt/trn_rl_repo')
import numpy as np

B, C, H, W = 32, 3, 512, 512
KP = 16                 # patch size
NB = H // KP            # 32 patch blocks per side
N = NB * NB             # 1024 patches
PD = KP * KP            # 256 positions per patch (single channel)
N_CORES = 8
IPC = B // N_CORES      # 4 images per core
HB = B // 2             # host pipeline half-batch
IPH = IPC // 2          # images per core per half
THETA = 3.0 * 2.0 ** -10   # re-rank gate: 1.5x the 2^-9 fp16 gap bound

# With half-split inputs, core c computes images (2c, 2c+1, 16+2c, 16+2c+1)
# as its output rows 4c..4c+3; _ROW_OF_IMG[i] is the gathered-output row
# holding image i.
_ROW_OF_IMG = np.empty(B, np.int64)
for _c in range(N_CORES):
    for _b in range(IPC):
        _img = 2 * _c + _b if _b < IPH else HB + 2 * _c + (_b - IPH)
        _ROW_OF_IMG[_img] = IPC * _c + _b

_CACHE = {}


def _build():
    import concourse.bass as bass
    import concourse.mybir as mybir
    from concourse.tile import TileContext

    F32 = mybir.dt.float32
    F16 = mybir.dt.float16
    U16 = mybir.dt.uint16

    nc = bass.Bass()
    qa_d = nc.declare_dram_parameter("qTa", [IPH, PD, N], F16, isOutput=False)
    qb_d = nc.declare_dram_parameter("qTb", [IPH, PD, N], F16, isOutput=False)
    ka_d = nc.declare_dram_parameter("kTa", [IPH, PD, N], F16, isOutput=False)
    kb_d = nc.declare_dram_parameter("kTb", [IPH, PD, N], F16, isOutput=False)
    am8_d = nc.declare_dram_parameter("am8", [IPC, N, 8], U16, isOutput=True)
    mx2_d = nc.declare_dram_parameter("mx2", [IPC, N, 2], F32, isOutput=True)
    an8_d = nc.declare_dram_parameter("an8", [IPC, N, 8], U16, isOutput=True)
    mn2_d = nc.declare_dram_parameter("mn2", [IPC, N, 2], F32, isOutput=True)

    am8_v = am8_d.rearrange("i (t p) e -> i t p e", p=128)
    mx2_v = mx2_d.rearrange("i (t p) e -> i t p e", p=128)
    an8_v = an8_d.rearrange("i (t p) e -> i t p e", p=128)
    mn2_v = mn2_d.rearrange("i (t p) e -> i t p e", p=128)

    with TileContext(nc) as tc:
        with (
            tc.tile_pool(name="qk", bufs=8) as qkp,
            tc.tile_pool(name="esb", bufs=4) as esbp,
            tc.tile_pool(name="tiny", bufs=8) as tinyp,
            tc.tile_pool(name="idx", bufs=8) as idxp,
            tc.tile_pool(name="ps", bufs=4, space="PSUM") as psp,
        ):
            for b in range(IPC):
                q_d = qa_d if b < IPH else qb_d
                k_d = ka_d if b < IPH else kb_d
                bi = b if b < IPH else b - IPH
                qlo = qkp.tile([128, N], F16, name="qlo", tag="qk")
                qhi = qkp.tile([128, N], F16, name="qhi", tag="qk")
                klo = qkp.tile([128, N], F16, name="klo", tag="qk")
                khi = qkp.tile([128, N], F16, name="khi", tag="qk")
                nc.sync.dma_start(out=qlo[:], in_=q_d[bi, 0:128, :])
                nc.scalar.dma_start(out=qhi[:], in_=q_d[bi, 128:256, :])
                nc.sync.dma_start(out=klo[:], in_=k_d[bi, 0:128, :])
                nc.scalar.dma_start(out=khi[:], in_=k_d[bi, 128:256, :])
                for mt in range(8):
                    esb = esbp.tile([128, N], F32, name="esb", tag="esb")
                    for nf in range(2):
                        pe = psp.tile([128, 512], F32, name="pe", tag="ps", space="PSUM")
                        nc.tensor.matmul(pe[:], qlo[:, 128 * mt:128 * (mt + 1)],
                                         klo[:, 512 * nf:512 * (nf + 1)], start=True, stop=False)
                        nc.tensor.matmul(pe[:], qhi[:, 128 * mt:128 * (mt + 1)],
                                         khi[:, 512 * nf:512 * (nf + 1)], start=False, stop=True)
                        nc.scalar.copy(esb[:, 512 * nf:512 * (nf + 1)], pe[:])
                    mx = tinyp.tile([128, 8], F32, name="mx", tag="tiny")
                    ix = idxp.tile([128, 8], U16, name="ix", tag="idx")
                    nc.vector.max(mx[:], esb[:])
                    nc.vector.max_index(ix[:], mx[:], esb[:])
                    nc.sync.dma_start(out=am8_v[b, mt], in_=ix[:])
                    nc.sync.dma_start(out=mx2_v[b, mt], in_=mx[:, 0:2])
                    esn = esbp.tile([128, N], F32, name="esn", tag="esb")
                    nc.scalar.mul(esn[:], esb[:], -1.0)
                    mn = tinyp.tile([128, 8], F32, name="mn", tag="tiny")
                    inx = idxp.tile([128, 8], U16, name="inx", tag="idx")
                    nc.vector.max(mn[:], esn[:])
                    nc.vector.max_index(inx[:], mn[:], esn[:])
                    nc.scalar.dma_start(out=an8_v[b, mt], in_=inx[:])
                    nc.scalar.dma_start(out=mn2_v[b, mt], in_=mn[:, 0:2])

    # wait-splitting post-pass (walrus in this container allows 1 sync-wait/inst)
    for f in nc.m.functions:
        for blk in f.blocks:
            newlist = []
            for i in blk.instructions:
                si = i.sync_info
                if si is not None and len(si.on_wait) > 1:
                    waits = list(si.on_wait)
                    keep = waits[-1:]
                    rest = waits[:-1]
                    for j, wchunk in enumerate(rest):
                        nop = mybir.InstNoOp(name=f"{i.name}-ws-{j}", ins=[], outs=[])
                        nop.engine = i.engine
                        nop.sync_info = mybir.SyncInfo(on_wait=[wchunk], on_update=[])
                        newlist.append(nop)
                    si.on_wait = keep
                newlist.append(i)
            blk.instructions[:] = newlist
    return nc


def _get_runner():
    """Cached jitted SPMD runner + input sharding for async device_put."""
    if "runner" in _CACHE:
        return _CACHE["runner"]
    import jax
    import concourse.mybir as mybir
    from concourse import bass2jax
    from jax.experimental.shard_map import shard_map
    from jax.sharding import Mesh, PartitionSpec, NamedSharding

    nc = _build()
    bass2jax.install_neuronx_cc_hook()

    partition_name = nc.partition_id_tensor.name if nc.partition_id_tensor else None
    in_names, out_names, out_avals = [], [], []
    for alloc in nc.m.functions[0].allocations:
        if not isinstance(alloc, mybir.MemoryLocationSet):
            continue
        name = alloc.memorylocations[0].name
        if alloc.kind == "ExternalInput":
            if name != partition_name:
                in_names.append(name)
        elif alloc.kind == "ExternalOutput":
            out_names.append(name)
            out_avals.append(jax.core.ShapedArray(tuple(alloc.tensor_shape),
                                                  mybir.dt.np(alloc.dtype)))
    n_params = len(in_names)
    n_outs = len(out_avals)
    all_in_names = list(in_names) + list(out_names)
    if partition_name is not None:
        all_in_names.append(partition_name)

    def _body(*args):
        operands = list(args)
        if partition_name is not None:
            operands.append(bass2jax.partition_id_tensor())
        outs = bass2jax._bass_exec_p.bind(
            *operands,
            out_avals=tuple(out_avals),
            in_names=tuple(all_in_names),
            out_names=tuple(out_names),
            lowering_input_output_aliases=(),
            sim_require_finite=True,
            sim_require_nnan=True,
            nc=nc,
        )
        return tuple(outs)

    devices = jax.devices()[:N_CORES]
    mesh = Mesh(np.asarray(devices), ("core",))
    donate = tuple(range(n_params, n_params + n_outs))
    sharded = jax.jit(
        shard_map(_body, mesh=mesh,
                  in_specs=(PartitionSpec("core"),) * (n_params + n_outs),
                  out_specs=(PartitionSpec("core"),) * n_outs,
                  check_rep=False),
        donate_argnums=donate, keep_unused=True,
    )
    sharding = NamedSharding(mesh, PartitionSpec("core"))
    zero_shapes = [(N_CORES * a.shape[0], *a.shape[1:]) for a in out_avals]
    zero_dtypes = [a.dtype for a in out_avals]
    make_zeros = jax.jit(
        lambda: tuple(jax.numpy.zeros(s, d) for s, d in zip(zero_shapes, zero_dtypes)),
        out_shardings=(sharding,) * n_outs,
    )
    runner = (sharded, make_zeros, in_names, out_names, sharding, jax)
    _CACHE["runner"] = runner
    return runner


def _conv1x1(feat, w, bias):
    # feat: [b,C,H,W], w: [C], bias: scalar -> [b,H,W]
    nb = feat.shape[0]
    out = np.einsum('bcx,c->bx', feat.reshape(nb, C, H * W), w.astype(np.float32))
    out += np.float32(bias)
    return out.reshape(nb, H, W)


def _pack_T(img):
    # [b,H,W] -> [b, (r,s)=256, (hb,wb)=1024]
    nb = img.shape[0]
    v = img.reshape(nb, NB, KP, NB, KP)
    return np.ascontiguousarray(v.transpose(0, 2, 4, 1, 3)).reshape(nb, PD, N)


def _gather_fold(x, idx):
    # x: [B,C,H,W] natural; idx: [B, N] patch indices -> [B,C,H,W]
    hbs = (idx // NB).reshape(B, NB, NB)
    wbs = (idx % NB).reshape(B, NB, NB)
    out = np.empty((B, C, H, W), np.float32)
    for b in range(B):
        ov = out[b].reshape(C, NB, KP, NB, KP).transpose(1, 3, 0, 2, 4)
        ov[...] = x[b].reshape(C, NB, KP, NB, KP)[:, hbs[b], :, wbs[b], :]
    return out


def _prep_k(feat_half, wk, bk):
    """conv + pack + normalize-fused-fp16-cast for one image half."""
    k = _conv1x1(feat_half, wk, bk)
    kT = _pack_T(k)
    knrm = np.maximum(np.sqrt(np.einsum('bpn,bpn->bn', kT, kT)), np.float32(1e-12))
    k16 = np.empty(kT.shape, np.float16)
    np.multiply(kT, (np.float32(1.0) / knrm)[:, None, :], out=k16, casting='unsafe')
    return kT, knrm, k16


def _rerank(sel, top8, gap, qn, qT, kT, knrm, pick_min):
    """Exact fp32 re-rank of gap-ambiguous queries; edits sel in place.

    kT is the RAW fp32 k; candidates are normalized here with the same
    fp32 elementwise divide the reference uses."""
    ib, im = np.nonzero(gap <= THETA * qn)
    if ib.size == 0:
        return 0
    cand = top8[ib, im].astype(np.int64)            # [M,8]
    kv = kT[ib[:, None], :, cand]                   # [M,8,256] raw fp32 k
    kv /= knrm[ib[:, None], cand][:, :, None]
    qv = qT[ib, :, im]                              # [M,256]
    e = np.einsum('mcp,mp->mc', kv, qv)
    best = e.argmin(1) if pick_min else e.argmax(1)
    sel[ib, im] = cand[np.arange(ib.size), best]
    return ib.size


def kernel(**inputs) -> np.ndarray:
    feat_edit = np.asarray(inputs["feat_edit"], dtype=np.float32)
    feat_ori = np.asarray(inputs["feat_ori"], dtype=np.float32)
    x1 = np.asarray(inputs["x1"], dtype=np.float32)
    wq = np.asarray(inputs["wq"], dtype=np.float32).ravel()
    bq = np.asarray(inputs["bq"], dtype=np.float32).ravel()[0]
    wk = np.asarray(inputs["wk"], dtype=np.float32).ravel()
    bk = np.asarray(inputs["bk"], dtype=np.float32).ravel()[0]
    gamma2 = np.float32(np.asarray(inputs["gamma2"], dtype=np.float32).ravel()[0])
    with_x2 = bool(gamma2 != 0.0)

    sharded, make_zeros, in_names, out_names, sharding, jax = _get_runner()

    # pre: conv + pack + fp16 cast per half-batch; each half's wire transfer
    # (async device_put) starts as soon as it is ready, overlapping the rest
    dev = {}
    qTa = _pack_T(_conv1x1(feat_edit[:HB], wq, bq))
    dev["qTa"] = jax.device_put(qTa.astype(np.float16), sharding)
    qTb = _pack_T(_conv1x1(feat_edit[HB:], wq, bq))
    dev["qTb"] = jax.device_put(qTb.astype(np.float16), sharding)
    kTa, knrma, k16a = _prep_k(feat_ori[:HB], wk, bk)
    dev["kTa"] = jax.device_put(k16a, sharding)
    kTb, knrmb, k16b = _prep_k(feat_ori[HB:], wk, bk)
    dev["kTb"] = jax.device_put(k16b, sharding)
    args = [dev[n] for n in in_names] + list(make_zeros())
    out_arrs = sharded(*args)  # async dispatch

    # overlap window: host work that doesn't need the device results
    qna = np.sqrt(np.einsum('bpn,bpn->bn', qTa, qTa))   # re-rank gate
    qnb = np.sqrt(np.einsum('bpn,bpn->bn', qTb, qTb))

    am8, mx2 = jax.device_get((out_arrs[out_names.index("am8")],
                               out_arrs[out_names.index("mx2")]))
    am8 = am8[_ROW_OF_IMG]
    mx2 = mx2[_ROW_OF_IMG]
    am = am8[:, :, 0].astype(np.int64)
    gap = mx2[:, :, 0] - mx2[:, :, 1]
    _rerank(am[:HB], am8[:HB], gap[:HB], qna, qTa, kTa, knrma, pick_min=False)
    _rerank(am[HB:], am8[HB:], gap[HB:], qnb, qTb, kTb, knrmb, pick_min=False)
    out = _gather_fold(x1, am)
    if with_x2:
        an8, mn2 = jax.device_get((out_arrs[out_names.index("an8")],
                                   out_arrs[out_names.index("mn2")]))
        an8 = an8[_ROW_OF_IMG]
        mn2 = mn2[_ROW_OF_IMG]
        an = an8[:, :, 0].astype(np.int64)
        gapn = mn2[:, :, 0] - mn2[:, :, 1]
        _rerank(an[:HB], an8[:HB], gapn[:HB], qna, qTa, kTa, knrma, pick_min=True)
        _rerank(an[HB:], an8[HB:], gapn[HB:], qnb, qTb, kTb, knrmb, pick_min=True)
        out += gamma2 * _gather_fold(np.asarray(inputs["x2"], dtype=np.float32), an)
    return out
